# revision 12
# baseline (speedup 1.0000x reference)
"""GQA attention (B=2, S=2048, DIM=4096, H=32, KV=8, HD=128) with interleaved
RoPE + causal mask, distributed over 8 TRN2 NeuronCores.

Sharding: tensor-parallel over KV groups. Core c owns kv-group c (4 query
heads + 1 kv head): it projects Q/K/V for its group over ALL tokens (both
batches), runs causal attention locally (fully uniform program across cores),
then one AllToAll re-shards the attention output from head-major to
token-major, and each core computes the output projection for its 512-token
slice against the full wo. No all-reduce.

Matmuls run as float32r (TF32 datapath, 1 cycle/row at moving>=256).
RoPE is applied in feature-major layout via a pair-swap permutation matmul
plus precomputed partition-pair cos/sin tables (softmax scale folded into the
Q tables). Softmax denominators come from a DVE accumulation + ones-matmul
partition reduction; normalization is broadcast back via a K=1 ones-matmul.
"""
import sys
sys.path.insert(0, "/opt/trn_rl_repo")
import numpy as np

B, S, DIM = 2, 2048, 4096
H, KV, HD = 32, 8, 128
SCALE = HD ** -0.5
NC = 8
NT = B * S            # 4096 flat tokens
TC = 512              # token chunk for projections
NEG = np.float32(np.finfo(np.float32).min)

_CACHE = {}


def _build():
    import concourse.bacc as bacc
    import concourse.tile as tile
    from concourse import mybir

    F32 = mybir.dt.float32
    F32R = mybir.dt.float32r
    BF16 = mybir.dt.bfloat16
    EXP = mybir.ActivationFunctionType.Exp

    nc = bacc.Bacc("TRN2", target_bir_lowering=False, num_devices=NC)

    def param(name, shape):
        return nc.declare_dram_parameter(name, shape, F32, isOutput=False)

    xt = param("xt", [DIM, NT])          # x flattened+transposed (full)
    wq_s = param("wq_s", [DIM, 512])     # my 4 heads' wq columns
    wk_s = param("wk_s", [DIM, 128])
    wv_s = param("wv_s", [DIM, 128])
    wo = param("wo", [DIM, DIM])
    cq = param("cq", [128, NT])          # RoPE tables, partition-pair layout
    sq = param("sq", [128, NT])          # (cq/sq scaled by 1/sqrt(HD))
    ck = param("ck", [128, NT])
    sk = param("sk", [128, NT])
    maskt = param("maskt", [2, 128, 256])  # transposed causal diag block, 2 k-tiles
    pswap = param("pswap", [128, 128])   # pair-swap permutation
    ident = param("ident", [128, 128])   # identity (PE transpose)
    ones = param("ones", [128, 128])
    out_ext = nc.declare_dram_parameter("out", [512, DIM], F32, isOutput=True)
    import os
    DBG = bool(os.environ.get("KERNEL_DEBUG"))
    if DBG:
        dbg_qt = nc.declare_dram_parameter("dbg_qt", [128, NT], F32, isOutput=True)
        dbg_kt = nc.declare_dram_parameter("dbg_kt", [128, NT], F32, isOutput=True)
        dbg_v = nc.declare_dram_parameter("dbg_v", [128, NT], F32, isOutput=True)
        dbg_a2ain = nc.declare_dram_parameter("dbg_a2ain", [NC, 512, 512], F32, isOutput=True)

    with tile.TileContext(nc) as tc:
        import contextlib
        with contextlib.ExitStack() as ctx:
            dram = ctx.enter_context(tc.tile_pool(name="dram", bufs=1, space="DRAM"))
            a2a_in = dram.tile([NC, 512, 512], F32, name="a2a_in")
            a2a_out = dram.tile([NC, 512, 512], F32, name="a2a_out")

            consts = ctx.enter_context(tc.tile_pool(name="consts", bufs=1))
            pswap_sb = consts.tile([128, 128], F32R)
            ident_f32 = consts.tile([128, 128], F32)
            ident_sb = consts.tile([128, 128], BF16)
            ones_sb = consts.tile([128, 128], F32R)
            maskt0_sb = consts.tile([128, 256], F32)
            maskt1_sb = consts.tile([128, 256], F32)
            nc.sync.dma_start(out=pswap_sb, in_=pswap[:, :].bitcast(F32R))
            nc.sync.dma_start(out=ident_f32, in_=ident[:, :])
            nc.vector.tensor_copy(ident_sb[:, :], ident_f32[:, :])
            nc.sync.dma_start(out=ones_sb, in_=ones[:, :].bitcast(F32R))
            nc.sync.dma_start(out=maskt0_sb, in_=maskt[0, :, :])
            nc.sync.dma_start(out=maskt1_sb, in_=maskt[1, :, :])

            # persistent per-core tensors (live through attention only)
            ph12 = contextlib.ExitStack()
            qt_pool = ph12.enter_context(tc.tile_pool(name="qt", bufs=4))
            ktv_pool = ph12.enter_context(tc.tile_pool(name="ktv", bufs=2))
            qt_sb = [qt_pool.tile([128, NT], F32R, tag="qt", name=f"qt{i}") for i in range(4)]
            kt_sb = ktv_pool.tile([128, NT], F32R, tag="ktv", name="kt_sb")
            v_sb = ktv_pool.tile([128, NT], BF16, tag="vtv", name="v_sb")

            # ---------------- Phase 1: QKV projection + RoPE ----------------
            with (
                tc.tile_pool(name="xw", bufs=4) as xw,
                tc.tile_pool(name="cs", bufs=2) as csp,
                tc.tile_pool(name="rope", bufs=4) as rope,
                tc.tile_pool(name="pj_ps", bufs=4, space="PSUM") as pj_ps,
                tc.tile_pool(name="kv_ps", bufs=2, space="PSUM") as kv_ps,
                tc.tile_pool(name="sw_ps", bufs=2, space="PSUM") as sw_ps,
            ):
                for tcb in range(NT // TC):
                    ts = slice(tcb * TC, (tcb + 1) * TC)
                    qps = [pj_ps.tile([128, TC], F32, tag="qps", name=f"qps{i}") for i in range(4)]
                    kps = kv_ps.tile([128, TC], F32, tag="kvps")
                    vps = kv_ps.tile([128, TC], F32, tag="kvps")
                    for d in range(DIM // 128):
                        ds_ = slice(d * 128, (d + 1) * 128)
                        xt_t = xw.tile([128, TC], F32R, tag="xt_t")
                        wq_t = xw.tile([128, 512], F32R, tag="wq_t")
                        wkv_t = xw.tile([128, 256], F32R, tag="wkv_t")
                        nc.sync.dma_start(out=xt_t, in_=xt[ds_, ts].bitcast(F32R))
                        nc.sync.dma_start(out=wq_t, in_=wq_s[ds_, :].bitcast(F32R))
                        nc.sync.dma_start(out=wkv_t[:, 0:128],
                                          in_=wk_s[ds_, :].bitcast(F32R))
                        nc.sync.dma_start(out=wkv_t[:, 128:256],
                                          in_=wv_s[ds_, :].bitcast(F32R))
                        st = (d == 0)
                        sp = (d == DIM // 128 - 1)
                        for f in range(4):
                            nc.tensor.matmul(qps[f][:, :], wq_t[:, f * 128:(f + 1) * 128],
                                             xt_t[:, :], start=st, stop=sp)
                        nc.tensor.matmul(kps[:, :], wkv_t[:, 0:128], xt_t[:, :],
                                         start=st, stop=sp)
                        nc.tensor.matmul(vps[:, :], wkv_t[:, 128:256], xt_t[:, :],
                                         start=st, stop=sp)

                    cq_t = csp.tile([128, TC], F32, tag="cq_t")
                    sq_t = csp.tile([128, TC], F32, tag="sq_t")
                    ck_t = csp.tile([128, TC], F32, tag="ck_t")
                    sk_t = csp.tile([128, TC], F32, tag="sk_t")
                    nc.scalar.dma_start(out=cq_t, in_=cq[:, ts])
                    nc.scalar.dma_start(out=sq_t, in_=sq[:, ts])
                    nc.scalar.dma_start(out=ck_t, in_=ck[:, ts])
                    nc.scalar.dma_start(out=sk_t, in_=sk[:, ts])

                    # RoPE(t) = t*C + (P@t)*S ; write into persistent qt/kt
                    for f in range(4):
                        raw = rope.tile([128, TC], F32R, tag="raw")
                        nc.vector.tensor_copy(raw[:, :], qps[f][:, :])
                        swp = sw_ps.tile([128, TC], F32, tag="swp")
                        nc.tensor.matmul(swp[:, :], pswap_sb[:, :], raw[:, :],
                                         start=True, stop=True)
                        t1 = rope.tile([128, TC], F32, tag="t1")
                        nc.vector.tensor_mul(t1[:, :], raw[:, :], cq_t[:, :])
                        t2 = rope.tile([128, TC], F32, tag="t2")
                        nc.vector.tensor_mul(t2[:, :], swp[:, :], sq_t[:, :])
                        nc.vector.tensor_add(qt_sb[f][:, ts], t1[:, :], t2[:, :])
                    kraw = rope.tile([128, TC], F32R, tag="raw")
                    nc.vector.tensor_copy(kraw[:, :], kps[:, :])
                    kswp = sw_ps.tile([128, TC], F32, tag="swp")
                    nc.tensor.matmul(kswp[:, :], pswap_sb[:, :], kraw[:, :],
                                     start=True, stop=True)
                    t1 = rope.tile([128, TC], F32, tag="t1")
                    nc.vector.tensor_mul(t1[:, :], kraw[:, :], ck_t[:, :])
                    t2 = rope.tile([128, TC], F32, tag="t2")
                    nc.vector.tensor_mul(t2[:, :], kswp[:, :], sk_t[:, :])
                    nc.vector.tensor_add(kt_sb[:, ts], t1[:, :], t2[:, :])

                    # V: evacuate then PE-transpose to token-major tiles
                    vraw = rope.tile([128, TC], BF16, tag="vraw")
                    nc.vector.tensor_copy(vraw[:, :], vps[:, :])
                    for tt in range(4):
                        tps = sw_ps.tile([128, 128], BF16, tag="swp", name="tps")
                        nc.tensor.transpose(tps[:, :],
                                            vraw[:, tt * 128:(tt + 1) * 128],
                                            ident_sb[:, :])
                        gtt = tcb * 4 + tt
                        nc.vector.tensor_copy(v_sb[:, gtt * 128:(gtt + 1) * 128],
                                              tps[:, :])

            # ---------------- Phase 2: causal attention (local group) -------
            with (
                tc.tile_pool(name="att", bufs=6) as att,
                tc.tile_pool(name="dacc", bufs=8) as daccp,
                tc.tile_pool(name="osb", bufs=4) as osbp,
                tc.tile_pool(name="sp_ps", bufs=2, space="PSUM") as sp_ps,
                tc.tile_pool(name="op_ps", bufs=4, space="PSUM") as op_ps,
                tc.tile_pool(name="dn_ps", bufs=1, space="PSUM") as dn_ps,
                tc.tile_pool(name="bc_ps", bufs=1, space="PSUM") as bc_ps,
            ):
                for b in range(B):
                    for qc in range(8):
                        q0 = 256 * qc
                        n_k = 2 * qc + 2
                        qsl = slice(b * S + q0, b * S + q0 + 256)
                        ops = [op_ps.tile([128, 256], F32, tag="ops", name=f"ops{i}") for i in range(4)]
                        daccs = [daccp.tile([128, 256], F32R, tag="dacc",
                                             name=f"dacc{i}") for i in range(4)]
                        for kt in range(n_k):
                            k0 = b * S + kt * 128
                            ktile = kt_sb[:, k0:k0 + 128]
                            vtile = v_sb[:, k0:k0 + 128]
                            for h in range(4):
                                sps = sp_ps.tile([128, 256], F32, tag="sps")
                                nc.tensor.matmul(sps[:, :], ktile,
                                                 qt_sb[h][:, qsl],
                                                 start=True, stop=True)
                                et = att.tile([128, 256], BF16, tag="et")
                                if kt >= 2 * qc:
                                    mt = maskt0_sb if kt == 2 * qc else maskt1_sb
                                    msk = att.tile([128, 256], F32, tag="msk")
                                    nc.vector.tensor_add(msk[:, :], sps[:, :],
                                                         mt[:, :])
                                    nc.scalar.activation(et[:, :], msk[:, :], EXP)
                                else:
                                    nc.scalar.activation(et[:, :], sps[:, :], EXP)
                                if kt == 0:
                                    nc.vector.tensor_copy(daccs[h][:, :], et[:, :])
                                else:
                                    nc.vector.tensor_add(daccs[h][:, :],
                                                         daccs[h][:, :], et[:, :])
                                nc.tensor.matmul(
                                    ops[h][:, :], vtile, et[:, :],
                                    start=(kt == 0), stop=(kt == n_k - 1))
                        s = b * 4 + qc // 2
                        tokoff = (qc % 2) * 256
                        for h in range(4):
                            dn = dn_ps.tile([1, 256], F32, tag="dn")
                            nc.tensor.matmul(dn[0:1, :], ones_sb[:, 0:1],
                                             daccs[h][:, :], start=True, stop=True)
                            rec = att.tile([1, 256], F32R, tag="rec")
                            with nc.allow_low_precision(reason="f32r recip feeds bcast matmul"):
                                nc.vector.reciprocal(rec[:, :], dn[:, :])
                            bcp = bc_ps.tile([128, 256], F32, tag="bcp")
                            nc.tensor.matmul(bcp[:, :], ones_sb[0:1, :], rec[:, :],
                                             start=True, stop=True)
                            bsb = att.tile([128, 256], F32, tag="bsb")
                            nc.vector.tensor_copy(bsb[:, :], bcp[:, :])
                            osb = osbp.tile([128, 256], F32, tag="osb")
                            nc.vector.tensor_mul(osb[:, :], ops[h][:, :], bsb[:, :])
                            nc.scalar.dma_start(
                                out=a2a_in[s, h * 128:(h + 1) * 128,
                                           tokoff:tokoff + 256],
                                in_=osb[:, :])

            if DBG:
                nc.scalar.dma_start(out=dbg_qt[:, :], in_=qt_sb[0][:, :].bitcast(F32))
                nc.scalar.dma_start(out=dbg_kt[:, :], in_=kt_sb[:, :].bitcast(F32))
                with tc.tile_pool(name="dbgp", bufs=1) as dbgp:
                    for i in range(8):
                        vconv = dbgp.tile([128, TC], F32, tag="vconv", name=f"vc{i}")
                        nc.vector.tensor_copy(vconv[:, :], v_sb[:, i * TC:(i + 1) * TC])
                        nc.scalar.dma_start(out=dbg_v[:, i * TC:(i + 1) * TC], in_=vconv[:, :])

            ph12.close()  # release qt/ktv SBUF before out-proj pools

            # ---------------- Phase 3: AllToAll -----------------------------
            nc.gpsimd.collective_compute(
                "AllToAll", mybir.AluOpType.bypass,
                replica_groups=[list(range(NC))],
                ins=[a2a_in.opt()], outs=[a2a_out.opt()],
            )

            if DBG:
                nc.scalar.dma_start(out=dbg_a2ain[:, :, :], in_=a2a_in[:, :, :])

            # ---------------- Phase 4: output projection --------------------
            with (
                tc.tile_pool(name="otp", bufs=32) as otp,
                tc.tile_pool(name="wop", bufs=4) as wop,
                tc.tile_pool(name="ysb", bufs=4) as ysbp,
                tc.tile_pool(name="y_ps", bufs=8, space="PSUM") as y_ps,
            ):
                ot_sb = [otp.tile([128, 512], F32R, tag="ot", name=f"ot{i}") for i in range(32)]
                for f in range(32):
                    p, fr = f // 4, (f % 4) * 128
                    nc.scalar.dma_start(out=ot_sb[f],
                                        in_=a2a_out[p, fr:fr + 128, :].bitcast(F32R))
                for dchunk in range(8):
                    dsl = slice(dchunk * 512, (dchunk + 1) * 512)
                    yps = [y_ps.tile([128, 512], F32, tag="yps", name=f"yps{i}") for i in range(4)]
                    for f in range(32):
                        wo_t = wop.tile([128, 512], F32R, tag="wo_t")
                        nc.sync.dma_start(out=wo_t,
                                          in_=wo[f * 128:(f + 1) * 128, dsl].bitcast(F32R))
                        for tt in range(4):
                            nc.tensor.matmul(yps[tt][:, :],
                                             ot_sb[f][:, tt * 128:(tt + 1) * 128],
                                             wo_t[:, :],
                                             start=(f == 0), stop=(f == 31))
                    for tt in range(4):
                        y_sb = ysbp.tile([128, 512], F32, tag="y_sb")
                        nc.vector.tensor_copy(y_sb[:, :], yps[tt][:, :])
                        nc.scalar.dma_start(
                            out=out_ext[tt * 128:(tt + 1) * 128, dsl],
                            in_=y_sb[:, :])
    nc.compile()
    return nc


def _host_prep(x, freqs_cos, freqs_sin, mask):
    xt = np.ascontiguousarray(x.reshape(NT, DIM).T)
    pos = np.arange(NT) % S

    def cs(scale):
        c = np.empty((128, NT), np.float32)
        s = np.empty((128, NT), np.float32)
        ct, st_ = freqs_cos[pos].T * scale, freqs_sin[pos].T
        c[0::2] = ct
        c[1::2] = ct
        s[0::2] = -st_ * scale
        s[1::2] = st_ * scale
        return np.ascontiguousarray(c), np.ascontiguousarray(s)

    cq_, sq_ = cs(np.float32(SCALE))
    ck_, sk_ = cs(np.float32(1.0))
    pswap = np.zeros((128, 128), np.float32)
    for i in range(128):
        pswap[i, i ^ 1] = 1.0
    maskt = np.ascontiguousarray(
        np.stack([mask[:256, 0:128].T, mask[:256, 128:256].T]))
    ident = np.eye(128, dtype=np.float32)
    ones = np.ones((128, 128), np.float32)
    return xt, cq_, sq_, ck_, sk_, pswap, maskt, ident, ones


def kernel(x, wq, wk, wv, wo, freqs_cos, freqs_sin, mask, positions):
    from concourse.bass_utils import run_bass_kernel_spmd

    x = np.asarray(x, np.float32)
    wq = np.asarray(wq, np.float32)
    wk = np.asarray(wk, np.float32)
    wv = np.asarray(wv, np.float32)
    wo = np.asarray(wo, np.float32)
    freqs_cos = np.asarray(freqs_cos, np.float32)
    freqs_sin = np.asarray(freqs_sin, np.float32)
    mask = np.asarray(mask, np.float32)

    if "nc" not in _CACHE:
        _CACHE["nc"] = _build()
    nc = _CACHE["nc"]

    xt, cq_, sq_, ck_, sk_, pswap, maskt, ident, ones = _host_prep(
        x, freqs_cos, freqs_sin, mask)

    in_maps = []
    for c in range(NC):
        in_maps.append({
            "xt": xt,
            "wq_s": np.ascontiguousarray(wq[:, c * 512:(c + 1) * 512]),
            "wk_s": np.ascontiguousarray(wk[:, c * 128:(c + 1) * 128]),
            "wv_s": np.ascontiguousarray(wv[:, c * 128:(c + 1) * 128]),
            "wo": wo,
            "cq": cq_, "sq": sq_, "ck": ck_, "sk": sk_,
            "maskt": maskt, "pswap": pswap, "ident": ident, "ones": ones,
        })

    res = run_bass_kernel_spmd(nc, in_maps, core_ids=list(range(NC)))
    out = np.empty((NT, DIM), np.float32)
    for c in range(NC):
        out[c * 512:(c + 1) * 512, :] = res.results[c]["out"]
    return out.reshape(B, S, DIM)
